# revision 35
# baseline (speedup 1.0000x reference)
"""CrossCovarianceAttn Trainium2 kernel.

Data-parallel over B=8 across 8 NeuronCores; each core runs the full model on
one batch element. All big matmuls run in fp32r (tf32-class precision, 1
cyc/row for moving dim >= 256); PE transposes stream a bf16 identity (1
cyc/row). Norms over the token dim come from Gram-matrix diagonals computed on
the PE; covariance C_h, Gq_h come from one fused matmul per (head, chunk)
against a strided [q_h|k_h] view, Gk_h from a narrow k-only matmul. qkv
weights stay in natural q|k order (no permutation copies). The projection
contracts over 6 full 128-row feature blocks (attn@v results are repacked
96-row-per-head -> 128-row blocks).

Startup is DMA-bound, so weight tiles are split per matmul column group
(w_qkT into 3, w_vT into 6) and DMA issue is ordered [w q-half, x tile0,
w k-half, v during tile-0 compute] so the first qk matmuls start as soon as
their 4 weight blocks land. The rk (token-norm of k) broadcast to the softmax
free dim runs on the PE (transpose + rank-1 broadcast matmuls) instead of a
DRAM round trip, and vT tiles for the attn@v phase are prefetched during
phase 1.
"""
import os
import sys

sys.path.insert(0, "/opt/trn_rl_repo")

import numpy as np

import concourse.bass as bass
import concourse.mybir as mybir
import concourse.tile as tile
from concourse import bacc
from concourse.bass_utils import run_bass_kernel_spmd
from concourse.masks import make_identity

FP32 = mybir.dt.float32
FP32R = mybir.dt.float32r
BF16 = mybir.dt.bfloat16

N_TOK = 4096
C = 768
H = 8
HD = 96
C3 = 3 * C
TOK_TILE = 512
N_TILES = N_TOK // TOK_TILE
CHUNKS = TOK_TILE // 128
KK = C // 128
EPS = 1e-12

_CACHED_NC = None


def build_nc():
    nc = bacc.Bacc("TRN2", target_bir_lowering=False, debug=False, num_devices=8)

    x_d = nc.dram_tensor("x", (N_TOK, C), FP32, kind="ExternalInput").ap()
    wqkv_d = nc.dram_tensor("w_qkv", (C3, C), FP32, kind="ExternalInput").ap()
    temp_d = nc.dram_tensor("temperature", (H, 1, 1), FP32, kind="ExternalInput").ap()
    wproj_d = nc.dram_tensor("w_proj", (C, C), FP32, kind="ExternalInput").ap()
    bproj_d = nc.dram_tensor("b_proj", (C,), FP32, kind="ExternalInput").ap()
    out_d = nc.dram_tensor("out", (N_TOK, C), FP32, kind="ExternalOutput").ap()

    with tile.TileContext(nc) as tc:
        _build(tc, nc, x_d, wqkv_d, temp_d, wproj_d, bproj_d, out_d)
    nc.compile()
    return nc


def _build(tc, nc, x_d, wqkv_d, temp_d, wproj_d, bproj_d, out_d):
    import contextlib

    ctx = contextlib.ExitStack()
    with ctx:
        singles = ctx.enter_context(tc.tile_pool(name="singles", bufs=1))
        dram = ctx.enter_context(tc.tile_pool(name="dram", bufs=1, space="DRAM"))
        ps_tr = ctx.enter_context(tc.tile_pool(name="ps_tr", bufs=2, space="PSUM"))
        xin = ctx.enter_context(tc.tile_pool(name="xin", bufs=5))
        xinb = ctx.enter_context(tc.tile_pool(name="xinb", bufs=8))
        vtp = ctx.enter_context(tc.tile_pool(name="vtp", bufs=3))
        wpload = ctx.enter_context(tc.tile_pool(name="wpload", bufs=2))
        wpp = ctx.enter_context(tc.tile_pool(name="wpp", bufs=1))

        # bf16 identity: transposes stream the identity as the moving operand,
        # so a 16-bit identity runs them at 1 cyc/row (fp32 would be 2).
        ident = singles.tile([128, 128], BF16)
        make_identity(nc, ident)
        identf = singles.tile([96, 96], FP32)
        make_identity(nc, identf)

        warm = singles.tile([1, 1], FP32)
        nc.vector.memset(warm, 0.5)
        nc.scalar.activation(warm, warm, mybir.ActivationFunctionType.Exp)
        b_all = singles.tile([128, C], FP32)
        temp_all = singles.tile([HD, H], FP32)

        # cg layout per head: [0:96] Gq, [96:192] C, [192:288] Gk
        # (no memset: the tile-0 flush copies instead of accumulating)
        cg_accum = singles.tile([HD, H, 288], FP32)
        scr96 = singles.tile([HD, HD], FP32)
        sq = singles.tile([HD, 2, H], FP32)
        attnT = singles.tile([HD, H, HD], BF16)
        ones1f = singles.tile([1, HD], FP32)
        nc.vector.memset(ones1f, 1.0)
        ones1 = singles.tile([1, HD], FP32R)
        nc.vector.tensor_copy(ones1, ones1f)
        ones_col = singles.tile([1, 128], BF16)
        nc.vector.memset(ones_col, 1.0)
        b_rowb = singles.tile([1, C], BF16)
        UINT32 = mybir.dt.uint32
        rsq_k = singles.tile([HD, 2, H], UINT32)
        nc.vector.memset(rsq_k, 0x5F3759DF)
        rsq_s = singles.tile([HD, 2, 2, H], FP32)

        vT_dram = dram.tile([C, N_TOK], BF16)
        vt_tiles = {}

        def load_vt(t):
            t0 = t * TOK_TILE
            vT_t = vtp.tile([HD, H, TOK_TILE], BF16, name="vT_t")
            nc.gpsimd.dma_start(
                vT_t,
                vT_dram[:, t0:t0 + TOK_TILE].rearrange(
                    "(h d) n -> d h n", h=H))
            vt_tiles[t] = vT_t

        # ---------------- phase 0: qkv weight prep ----------------
        with tc.tile_pool(name="wload", bufs=3) as wload, \
             tc.tile_pool(name="wqk_pool", bufs=1) as wqk_pool:
            # per-512-col qk weight tiles so qk matmul group p only waits on
            # its own 4 source blocks; per-128-col v tiles likewise
            w_qkT = [wqk_pool.tile([128, KK, 512], BF16, name=f"w_qkT{p}")
                     for p in range(3)]
            w_vT = [wqk_pool.tile([128, KK, 128], BF16, name=f"w_vT{m}")
                    for m in range(6)]

            def prep_w_block(m):
                w_blk = wload.tile([128, C], FP32, name="w_blk")
                nc.sync.dma_start(w_blk, wqkv_d[m * 128:(m + 1) * 128, :])
                wbf = wload.tile([128, C], BF16, name="wbf")
                if m % 2 == 0:
                    nc.vector.tensor_copy(wbf, w_blk)
                else:
                    nc.scalar.copy(wbf, w_blk)
                for kk in range(KK):
                    tps = ps_tr.tile([128, 128], BF16, name="tps", tag="tr")
                    nc.tensor.transpose(
                        tps, wbf[:, kk * 128:(kk + 1) * 128], ident)
                    if m < 12:
                        dst = w_qkT[m // 4][:, kk, (m % 4) * 128:
                                            (m % 4) * 128 + 128]
                    else:
                        dst = w_vT[m - 12][:, kk, :]
                    if (m * KK + kk) % 2 == 0:
                        nc.vector.tensor_copy(dst, tps)
                    else:
                        nc.scalar.copy(dst, tps)

            def load_x_tile(t):
                t0 = t * TOK_TILE
                xcs = []
                for c in range(CHUNKS):
                    x_c = xin.tile([128, C], FP32, name="x_c")
                    nc.sync.dma_start(
                        x_c, x_d[t0 + c * 128: t0 + (c + 1) * 128, :])
                    xbf = xinb.tile([128, C], BF16, name="xbf")
                    if c % 2 == 0:
                        nc.vector.tensor_copy(xbf, x_c)
                    else:
                        nc.scalar.copy(xbf, x_c)
                    xcs.append(xbf)
                return xcs

            for m in range(4):
                prep_w_block(m)
            xcs_next = load_x_tile(0)
            nc.gpsimd.dma_start(
                b_all, bass.AP(tensor=bproj_d.tensor, offset=bproj_d.offset,
                               ap=[[0, 128], [1, C]]))
            nc.vector.tensor_copy(b_rowb, b_all[0:1, :])
            nc.gpsimd.dma_start(
                temp_all, bass.AP(tensor=temp_d.tensor, offset=temp_d.offset,
                                  ap=[[0, HD], [1, H]]))
            for m in range(4, 12):
                prep_w_block(m)

            # w_projT block m rows: [0:96] = head-m features 96m..96m+96,
            # [96:128] = 32-feature unit 576+32m (heads 6/7). The attn@v
            # repack writes heads at aligned offsets (0 or 96), and the
            # w_proj load gathers its contraction columns in this permuted
            # order (two strided DMAs per 128-row block).
            w_projT = wpp.tile([128, KK, C], BF16)

            def prep_wproj():
                for n in range(KK):
                    wp_blk = wpload.tile([128, C], FP32, name="wp_blk")
                    wpv = wp_blk.rearrange("p (m j) -> p m j", m=KK)
                    wsrc = wproj_d[n * 128:(n + 1) * 128, :]
                    nc.sync.dma_start(
                        wpv[:, :, 0:96],
                        wsrc[:, 0:576].rearrange("p (m j) -> p m j", j=96))
                    nc.sync.dma_start(
                        wpv[:, :, 96:128],
                        wsrc[:, 576:768].rearrange("p (m j) -> p m j", j=32))
                    wpbf = wpload.tile([128, C], BF16, name="wpbf")
                    nc.scalar.copy(wpbf, wp_blk)
                    for m in range(KK):
                        tps2 = ps_tr.tile([128, 128], BF16, name="tps2",
                                          tag="tr")
                        nc.tensor.transpose(
                            tps2, wpbf[:, m * 128:(m + 1) * 128], ident)
                        if (n * KK + m) % 2 == 0:
                            nc.vector.tensor_copy(
                                w_projT[:, m, n * 128:(n + 1) * 128], tps2)
                        else:
                            nc.scalar.copy(
                                w_projT[:, m, n * 128:(n + 1) * 128], tps2)

            # ---------------- phase 1 ----------------
            with tc.tile_pool(name="xtp", bufs=2) as xtp, \
                 tc.tile_pool(name="qkp", bufs=2) as qkp, \
                 tc.tile_pool(name="vtsb", bufs=2) as vtsb, \
                 tc.tile_pool(name="ps_mm", bufs=6, space="PSUM") as ps_mm:
                def tr_group(xT_t, xcs, kk):
                    xps = ps_tr.tile([128, TOK_TILE], BF16, name="xps",
                                     tag="tr")
                    for c in range(CHUNKS):
                        nc.tensor.transpose(
                            xps[:, c * 128:(c + 1) * 128],
                            xcs[c][:, kk * 128:(kk + 1) * 128], ident)
                    nc.vector.tensor_copy(xT_t[:, kk, :], xps)

                xT_next = xtp.tile([128, KK, TOK_TILE], BF16, name="xT_t")
                for kk in range(KK):
                    tr_group(xT_next, xcs_next, kk)
                for t in range(N_TILES):
                    xT_t = xT_next
                    qk_t = qkp.tile([128, CHUNKS, 1536], BF16, name="qk_t")
                    if t + 1 < N_TILES:
                        xcs = xcs_next
                        xcs_next = load_x_tile(t + 1)

                    # qk = xT.T @ w_qkT (token-major, natural q|k cols).
                    # p-outer: at startup group p can run while the weight
                    # blocks for p+1 are still in flight.
                    for p in range(3):
                        for c in range(CHUNKS):
                            mmps = ps_mm.tile([128, 512], FP32, name="mmps",
                                              tag="s")
                            for kk in range(KK):
                                nc.tensor.matmul(
                                    mmps, xT_t[:, kk, c * 128:(c + 1) * 128],
                                    w_qkT[p][:, kk, :],
                                    start=(kk == 0), stop=(kk == KK - 1))
                            if p == 1:
                                nc.scalar.copy(
                                    qk_t[:, c, p * 512:(p + 1) * 512], mmps)
                            else:
                                nc.vector.tensor_copy(
                                    qk_t[:, c, p * 512:(p + 1) * 512], mmps)

                    if t == 0:
                        # v weight prep streams in under tile-0's qk compute
                        for m in range(12, 18):
                            prep_w_block(m)

                    # covariance + Gram: per head, q-side gives [Gq | C]
                    # (rhs = strided [q_h|k_h] view, 192 free), k-side gives
                    # Gk only (96 free). One psum bank per head. Runs before
                    # vT so the flush adds drain off DVE while the PE chews
                    # the vT matmuls (no DVE backlog at the next tile's qk).
                    for h in range(H):
                        cg_ps = ps_mm.tile([HD, 288], FP32, name="cg_ps",
                                           tag="s")
                        for c in range(CHUNKS):
                            qh = qk_t[:, c, HD * h:HD * (h + 1)]
                            kh = qk_t[:, c, C + HD * h:C + HD * (h + 1)]
                            qk_pair = qk_t[:, c, :].rearrange(
                                "p (j e) -> p j e", j=2)[:, :,
                                                         HD * h:HD * (h + 1)]
                            nc.tensor.matmul(
                                cg_ps[:, 0:192], qh, qk_pair,
                                start=(c == 0), stop=False)
                            nc.tensor.matmul(
                                cg_ps[:, 192:288], kh, kh,
                                start=False, stop=(c == CHUNKS - 1))
                        if t == 0:
                            nc.vector.tensor_copy(cg_accum[:, h, :], cg_ps)
                        else:
                            nc.vector.tensor_add(
                                cg_accum[:, h, :], cg_ps, cg_accum[:, h, :])

                    # vT = w_vT.T @ xT (feature-major) -> DRAM. The next
                    # tile's transposes are interleaved between vT groups so
                    # each xps->xT copy has ~2.5us of PE cover (no psum-
                    # rotation stall at the tile boundary).
                    t0 = t * TOK_TILE
                    if t + 1 < N_TILES:
                        xT_next = xtp.tile([128, KK, TOK_TILE], BF16,
                                           name="xT_t")
                    vt_sb = vtsb.tile([128, KK, TOK_TILE], BF16, name="vt_sb")
                    for m in range(KK):
                        vps = ps_mm.tile([128, TOK_TILE], FP32, name="vps",
                                         tag="s")
                        for kk in range(KK):
                            nc.tensor.matmul(
                                vps, w_vT[m][:, kk, :],
                                xT_t[:, kk, :],
                                start=(kk == 0), stop=(kk == KK - 1))
                        nc.scalar.copy(vt_sb[:, m, :], vps)
                        if t + 1 < N_TILES:
                            tr_group(xT_next, xcs, m)
                    nc.scalar.dma_start(
                        vT_dram[:, t0:t0 + TOK_TILE].rearrange(
                            "(s p) n -> p s n", p=128),
                        vt_sb)
                    if t == N_TILES - 1:
                        # Gram diagonals, hidden under the t7 vT matmuls.
                        # k-side first: the PE rk-broadcast path depends only
                        # on the k norms, so it starts while DVE still
                        # processes the q side.
                        for h in range(H):
                            nc.vector.tensor_tensor(
                                scr96, cg_accum[:, h, 192:288], identf,
                                mybir.AluOpType.mult)
                            nc.vector.reduce_sum(
                                sq[:, 1, h:h + 1], scr96,
                                axis=mybir.AxisListType.X)
                        for h in range(H):
                            nc.vector.tensor_tensor(
                                scr96, cg_accum[:, h, 0:HD], identf,
                                mybir.AluOpType.mult)
                            nc.vector.reduce_sum(
                                sq[:, 0, h:h + 1], scr96,
                                axis=mybir.AxisListType.X)
                    if t < 2:
                        # prefetch attn@v inputs for the phase-2/3 handoff
                        load_vt(t)
                    if t == 6:
                        # w_projT prep hides under tile-7 compute
                        prep_wproj()


        # ---------------- phase 3 pools; w_projT prep emitted first so the
        # PE has work while the DVE/ACT-heavy phase 2 chain runs ----------
        with tc.tile_pool(name="otp", bufs=2) as otp, \
             tc.tile_pool(name="yp", bufs=2) as yp, \
             tc.tile_pool(name="ps_o", bufs=2, space="PSUM") as ps_o, \
             tc.tile_pool(name="ps_y", bufs=4, space="PSUM") as ps_y:
            # ---------------- phase 2: norms + softmax ----------------
            # rsqrt(Gram diag) via bit-trick seed + 2 Newton steps, all on
            # DVE: keeps Sqrt off ACT so Exp/Copy share one act table set
            # (no LoadActFuncSet on the critical chain).
            yv = rsq_s[:, :, 0, :]
            av = rsq_s[:, :, 1, :]
            nc.vector.tensor_scalar(
                yv.bitcast(UINT32), sq.bitcast(UINT32), 1, None,
                mybir.AluOpType.logical_shift_right)
            nc.vector.tensor_tensor(yv.bitcast(UINT32), rsq_k,
                                    yv.bitcast(UINT32),
                                    mybir.AluOpType.subtract)
            for _ in range(2):
                nc.vector.tensor_tensor(av, yv, yv, mybir.AluOpType.mult)
                nc.vector.tensor_tensor(av, av, sq, mybir.AluOpType.mult)
                nc.vector.tensor_scalar(av, av, -0.5, 1.5,
                                        mybir.AluOpType.mult,
                                        mybir.AluOpType.add)
                nc.vector.tensor_tensor(yv, yv, av, mybir.AluOpType.mult)
            rnorm = yv
            rq = singles.tile([HD, H], FP32)
            nc.vector.tensor_tensor(rq, rnorm[:, 0, :], temp_all,
                                    mybir.AluOpType.mult)

            # rk to the free dim via PE: per-head single-row transposes land
            # rk rows at partition 0; concatenate to [1, 768], then one rank-1
            # broadcast matmul (ones[1,96].T @ rk_flat) fills all partitions.
            rk_flat = singles.tile([1, H * HD], FP32R)
            for h in range(H):
                rkt_ps = ps_tr.tile([1, HD], FP32, name="rkt_ps", tag="tr")
                nc.tensor.transpose(rkt_ps, rnorm[:, 1, h:h + 1], identf)
                nc.vector.tensor_copy(
                    rk_flat[:, HD * h:HD * (h + 1)], rkt_ps)
            attL = singles.tile([HD, H, HD], FP32)
            for half in range(2):
                rb_ps = ps_tr.tile([HD, 4 * HD], FP32, name="rb_ps", tag="tr")
                nc.tensor.matmul(
                    rb_ps, ones1,
                    rk_flat[:, half * 4 * HD:(half + 1) * 4 * HD],
                    start=True, stop=True)
                nc.vector.tensor_tensor(
                    attL[:, 4 * half:4 * (half + 1), :],
                    cg_accum[:, 4 * half:4 * (half + 1), HD:2 * HD],
                    rb_ps.rearrange("p (h e) -> p h e", h=4),
                    mybir.AluOpType.mult)

            # per-head softmax pipeline: logits = C * rq[d] * rk[e] * temp,
            # |C*rq*rk| <= 1 (normalized correlation) so exp needs no max
            # subtraction. Exp folds rq in as a per-partition scale; the
            # 1/sum normalization is folded into the otsb copies later, so
            # each head's transpose (and attn@v) can start right after its
            # exp instead of after a batched chain.
            sea = singles.tile([HD, H, 1], FP32)
            rsea = singles.tile([HD, H, 1], FP32)
            for h in range(H):
                nc.scalar.activation(attL[:, h, :], attL[:, h, :],
                                     mybir.ActivationFunctionType.Exp,
                                     scale=rq[:, h:h + 1])
                nc.vector.reduce_sum(sea[:, h, :], attL[:, h, :],
                                     axis=mybir.AxisListType.X)
                nc.vector.reciprocal(rsea[:, h, :], sea[:, h, :])
                atps = ps_tr.tile([HD, HD], FP32, name="atps", tag="tr")
                nc.tensor.transpose(atps, attL[:, h, :], identf)
                if h % 2 == 0:
                    nc.vector.tensor_copy(attnT[:, h, :], atps)
                else:
                    nc.scalar.copy(attnT[:, h, :], atps)

            # ---------------- phase 3: attn@v + proj, sw-pipelined --------
            # attn@v psum rows (96/head) are repacked into 128-row feature
            # blocks so proj contracts over 6 full blocks instead of 8 ragged
            # head slices.
            def attnv_stage(t):
                vT_t = vt_tiles.pop(t)
                otsb = otp.tile([128, KK, TOK_TILE], BF16, name="otsb")
                flip = 0
                for h in range(H):
                    ops_ = ps_o.tile([HD, TOK_TILE], FP32, name="ops_")
                    nc.tensor.matmul(ops_, attnT[:, h, :], vT_t[:, h, :],
                                     start=True, stop=True)
                    if h < 6:
                        pieces = [(h, 0, 0, HD)]
                    else:
                        pieces = [(3 * (h - 6) + u, 96, 32 * u, 32)
                                  for u in range(3)]
                    for (bb, pp, s0, ln) in pieces:
                        dst = otsb[pp:pp + ln, bb, :]
                        src = ops_[s0:s0 + ln, :]
                        sc = rsea[s0:s0 + ln, h, :]
                        if flip % 2 == 0:
                            nc.vector.tensor_scalar(dst, src, sc, None,
                                                    mybir.AluOpType.mult)
                        else:
                            nc.scalar.mul(dst, src, sc)
                        flip += 1
                if t + 2 < N_TILES:
                    load_vt(t + 2)
                return otsb

            def proj_stage(t, otsb):
                t0 = t * TOK_TILE
                last = t == N_TILES - 1
                y_t = yp.tile([128, CHUNKS, C], FP32, name="y_t")
                for c in range(CHUNKS):
                    for (off, width) in ((0, 512), (512, 256)):
                        yps = ps_y.tile([128, 512], FP32, name="yps")
                        for m in range(KK):
                            nc.tensor.matmul(
                                yps[:, :width],
                                otsb[:, m, c * 128:(c + 1) * 128],
                                w_projT[:, m, off:off + width],
                                start=(m == 0), stop=(m == KK - 1))
                        nc.vector.tensor_tensor(
                            y_t[:, c, off:off + width], yps[:, :width],
                            b_all[:, off:off + width], mybir.AluOpType.add)
                        if last:
                            nc.sync.dma_start(
                                out_d[t0 + c * 128:t0 + (c + 1) * 128,
                                      off:off + width],
                                y_t[:, c, off:off + width])
                    if not last:
                        nc.sync.dma_start(
                            out_d[t0 + c * 128:t0 + (c + 1) * 128, :],
                            y_t[:, c, :])

            pend = None
            for t in range(N_TILES):
                cur = attnv_stage(t)
                if pend is not None:
                    proj_stage(*pend)
                pend = (t, cur)
            proj_stage(*pend)


def _get_nc():
    global _CACHED_NC
    if _CACHED_NC is None:
        _CACHED_NC = build_nc()
    return _CACHED_NC


def kernel(x, w_qkv, temperature, w_proj, b_proj):
    nc = _get_nc()
    x = np.ascontiguousarray(np.asarray(x, dtype=np.float32))
    in_maps = []
    for b in range(8):
        in_maps.append({
            "x": x[b],
            "w_qkv": np.asarray(w_qkv, dtype=np.float32),
            "temperature": np.asarray(temperature, dtype=np.float32),
            "w_proj": np.asarray(w_proj, dtype=np.float32),
            "b_proj": np.asarray(b_proj, dtype=np.float32),
        })
    res = run_bass_kernel_spmd(nc, in_maps, core_ids=list(range(8)))
    return np.stack([r["out"] for r in res.results], axis=0)


# revision 37
# speedup vs baseline: 1.0003x; 1.0003x over previous
"""CrossCovarianceAttn Trainium2 kernel.

Data-parallel over B=8 across 8 NeuronCores; each core runs the full model on
one batch element. All big matmuls run in fp32r (tf32-class precision, 1
cyc/row for moving dim >= 256); PE transposes stream a bf16 identity (1
cyc/row). Norms over the token dim come from Gram-matrix diagonals computed on
the PE; covariance C_h, Gq_h come from one fused matmul per (head, chunk)
against a strided [q_h|k_h] view, Gk_h from a narrow k-only matmul. qkv
weights stay in natural q|k order (no permutation copies). The projection
contracts over 6 full 128-row feature blocks (attn@v results are repacked
96-row-per-head -> 128-row blocks).

Startup is DMA-bound, so weight tiles are split per matmul column group
(w_qkT into 3, w_vT into 6) and DMA issue is ordered [w q-half, x tile0,
w k-half, v during tile-0 compute] so the first qk matmuls start as soon as
their 4 weight blocks land. The rk (token-norm of k) broadcast to the softmax
free dim runs on the PE (transpose + rank-1 broadcast matmuls) instead of a
DRAM round trip, and vT tiles for the attn@v phase are prefetched during
phase 1.
"""
import os
import sys

sys.path.insert(0, "/opt/trn_rl_repo")

import numpy as np

import concourse.bass as bass
import concourse.mybir as mybir
import concourse.tile as tile
from concourse import bacc
from concourse.bass_utils import run_bass_kernel_spmd
from concourse.masks import make_identity

FP32 = mybir.dt.float32
FP32R = mybir.dt.float32r
BF16 = mybir.dt.bfloat16

N_TOK = 4096
C = 768
H = 8
HD = 96
C3 = 3 * C
TOK_TILE = 512
N_TILES = N_TOK // TOK_TILE
CHUNKS = TOK_TILE // 128
KK = C // 128
EPS = 1e-12

_CACHED_NC = None


def build_nc():
    nc = bacc.Bacc("TRN2", target_bir_lowering=False, debug=False, num_devices=8)

    x_d = nc.dram_tensor("x", (N_TOK, C), FP32, kind="ExternalInput").ap()
    wqkv_d = nc.dram_tensor("w_qkv", (C3, C), FP32, kind="ExternalInput").ap()
    temp_d = nc.dram_tensor("temperature", (H, 1, 1), FP32, kind="ExternalInput").ap()
    wproj_d = nc.dram_tensor("w_proj", (C, C), FP32, kind="ExternalInput").ap()
    bproj_d = nc.dram_tensor("b_proj", (C,), FP32, kind="ExternalInput").ap()
    out_d = nc.dram_tensor("out", (N_TOK, C), FP32, kind="ExternalOutput").ap()

    with tile.TileContext(nc) as tc:
        _build(tc, nc, x_d, wqkv_d, temp_d, wproj_d, bproj_d, out_d)
    nc.compile()
    return nc


def _build(tc, nc, x_d, wqkv_d, temp_d, wproj_d, bproj_d, out_d):
    import contextlib

    ctx = contextlib.ExitStack()
    with ctx:
        singles = ctx.enter_context(tc.tile_pool(name="singles", bufs=1))
        dram = ctx.enter_context(tc.tile_pool(name="dram", bufs=1, space="DRAM"))
        ps_tr = ctx.enter_context(tc.tile_pool(name="ps_tr", bufs=2, space="PSUM"))
        xin = ctx.enter_context(tc.tile_pool(name="xin", bufs=5))
        xinb = ctx.enter_context(tc.tile_pool(name="xinb", bufs=8))
        vtp = ctx.enter_context(tc.tile_pool(name="vtp", bufs=3))
        wpload = ctx.enter_context(tc.tile_pool(name="wpload", bufs=2))
        wpp = ctx.enter_context(tc.tile_pool(name="wpp", bufs=1))

        # bf16 identity: transposes stream the identity as the moving operand,
        # so a 16-bit identity runs them at 1 cyc/row (fp32 would be 2).
        ident = singles.tile([128, 128], BF16)
        make_identity(nc, ident)
        identf = singles.tile([96, 96], FP32)
        make_identity(nc, identf)

        warm = singles.tile([1, 1], FP32)
        nc.vector.memset(warm, 0.5)
        nc.scalar.activation(warm, warm, mybir.ActivationFunctionType.Exp)
        b_all = singles.tile([128, C], FP32)
        temp_all = singles.tile([HD, H], FP32)

        # cg layout per head: [0:96] Gq, [96:192] C, [192:288] Gk
        # (no memset: the tile-0 flush copies instead of accumulating)
        cg_accum = singles.tile([HD, H, 288], FP32)
        scr96 = singles.tile([HD, HD], FP32)
        sq = singles.tile([HD, 2, H], FP32)
        attnT = singles.tile([HD, H, HD], BF16)
        ones1f = singles.tile([1, HD], FP32)
        nc.vector.memset(ones1f, 1.0)
        ones1 = singles.tile([1, HD], FP32R)
        nc.vector.tensor_copy(ones1, ones1f)
        ones_col = singles.tile([1, 128], BF16)
        nc.vector.memset(ones_col, 1.0)
        b_rowb = singles.tile([1, C], BF16)
        UINT32 = mybir.dt.uint32
        rsq_k = singles.tile([HD, 2, H], UINT32)
        nc.vector.memset(rsq_k, 0x5F3759DF)
        rsq_s = singles.tile([HD, 2, 2, H], FP32)

        vT_dram = dram.tile([C, N_TOK], BF16)
        vt_tiles = {}

        def load_vt(t):
            t0 = t * TOK_TILE
            vT_t = vtp.tile([HD, H, TOK_TILE], BF16, name="vT_t")
            nc.gpsimd.dma_start(
                vT_t,
                vT_dram[:, t0:t0 + TOK_TILE].rearrange(
                    "(h d) n -> d h n", h=H))
            vt_tiles[t] = vT_t

        # ---------------- phase 0: qkv weight prep ----------------
        with tc.tile_pool(name="wload", bufs=3) as wload, \
             tc.tile_pool(name="wqk_pool", bufs=1) as wqk_pool:
            # per-512-col qk weight tiles so qk matmul group p only waits on
            # its own 4 source blocks; per-128-col v tiles likewise
            w_qkT = [wqk_pool.tile([128, KK, 512], BF16, name=f"w_qkT{p}")
                     for p in range(3)]
            w_vT = [wqk_pool.tile([128, KK, 128], BF16, name=f"w_vT{m}")
                    for m in range(6)]

            def prep_w_block(m):
                w_blk = wload.tile([128, C], FP32, name="w_blk")
                nc.sync.dma_start(w_blk, wqkv_d[m * 128:(m + 1) * 128, :])
                wbf = wload.tile([128, C], BF16, name="wbf")
                if m % 2 == 0:
                    nc.vector.tensor_copy(wbf, w_blk)
                else:
                    nc.scalar.copy(wbf, w_blk)
                for kk in range(KK):
                    tps = ps_tr.tile([128, 128], BF16, name="tps", tag="tr")
                    nc.tensor.transpose(
                        tps, wbf[:, kk * 128:(kk + 1) * 128], ident)
                    if m < 12:
                        dst = w_qkT[m // 4][:, kk, (m % 4) * 128:
                                            (m % 4) * 128 + 128]
                    else:
                        dst = w_vT[m - 12][:, kk, :]
                    if (m * KK + kk) % 2 == 0:
                        nc.vector.tensor_copy(dst, tps)
                    else:
                        nc.scalar.copy(dst, tps)

            def load_x_tile(t):
                t0 = t * TOK_TILE
                xcs = []
                for c in range(CHUNKS):
                    x_c = xin.tile([128, C], FP32, name="x_c")
                    nc.sync.dma_start(
                        x_c, x_d[t0 + c * 128: t0 + (c + 1) * 128, :])
                    xbf = xinb.tile([128, C], BF16, name="xbf")
                    if c % 2 == 0:
                        nc.vector.tensor_copy(xbf, x_c)
                    else:
                        nc.scalar.copy(xbf, x_c)
                    xcs.append(xbf)
                return xcs

            for m in range(4):
                prep_w_block(m)
            xcs_next = load_x_tile(0)
            nc.gpsimd.dma_start(
                b_all, bass.AP(tensor=bproj_d.tensor, offset=bproj_d.offset,
                               ap=[[0, 128], [1, C]]))
            nc.vector.tensor_copy(b_rowb, b_all[0:1, :])
            nc.gpsimd.dma_start(
                temp_all, bass.AP(tensor=temp_d.tensor, offset=temp_d.offset,
                                  ap=[[0, HD], [1, H]]))
            for m in range(4, 12):
                prep_w_block(m)

            # w_projT block m rows: [0:96] = head-m features 96m..96m+96,
            # [96:128] = 32-feature unit 576+32m (heads 6/7). The attn@v
            # repack writes heads at aligned offsets (0 or 96), and the
            # w_proj load gathers its contraction columns in this permuted
            # order (two strided DMAs per 128-row block).
            w_projT = wpp.tile([128, KK, C], BF16)

            def prep_wproj():
                for n in range(KK):
                    wp_blk = wpload.tile([128, C], FP32, name="wp_blk")
                    wpv = wp_blk.rearrange("p (m j) -> p m j", m=KK)
                    wsrc = wproj_d[n * 128:(n + 1) * 128, :]
                    nc.sync.dma_start(
                        wpv[:, :, 0:96],
                        wsrc[:, 0:576].rearrange("p (m j) -> p m j", j=96))
                    nc.sync.dma_start(
                        wpv[:, :, 96:128],
                        wsrc[:, 576:768].rearrange("p (m j) -> p m j", j=32))
                    wpbf = wpload.tile([128, C], BF16, name="wpbf")
                    nc.scalar.copy(wpbf, wp_blk)
                    for m in range(KK):
                        tps2 = ps_tr.tile([128, 128], BF16, name="tps2",
                                          tag="tr")
                        nc.tensor.transpose(
                            tps2, wpbf[:, m * 128:(m + 1) * 128], ident)
                        if (n * KK + m) % 2 == 0:
                            nc.vector.tensor_copy(
                                w_projT[:, m, n * 128:(n + 1) * 128], tps2)
                        else:
                            nc.scalar.copy(
                                w_projT[:, m, n * 128:(n + 1) * 128], tps2)

            # ---------------- phase 1 ----------------
            with tc.tile_pool(name="xtp", bufs=2) as xtp, \
                 tc.tile_pool(name="qkp", bufs=2) as qkp, \
                 tc.tile_pool(name="vtsb", bufs=2) as vtsb, \
                 tc.tile_pool(name="ps_mm", bufs=6, space="PSUM") as ps_mm:
                def tr_group(xT_t, xcs, kk):
                    xps = ps_tr.tile([128, TOK_TILE], BF16, name="xps",
                                     tag="tr")
                    for c in range(CHUNKS):
                        nc.tensor.transpose(
                            xps[:, c * 128:(c + 1) * 128],
                            xcs[c][:, kk * 128:(kk + 1) * 128], ident)
                    nc.vector.tensor_copy(xT_t[:, kk, :], xps)

                xT_next = xtp.tile([128, KK, TOK_TILE], BF16, name="xT_t")
                for kk in range(KK):
                    tr_group(xT_next, xcs_next, kk)
                for t in range(N_TILES):
                    xT_t = xT_next
                    qk_t = qkp.tile([128, CHUNKS, 1536], BF16, name="qk_t")
                    if t + 1 < N_TILES:
                        xcs = xcs_next
                        xcs_next = load_x_tile(t + 1)

                    # qk = xT.T @ w_qkT (token-major, natural q|k cols).
                    # p-outer: at startup group p can run while the weight
                    # blocks for p+1 are still in flight.
                    for p in range(3):
                        for c in range(CHUNKS):
                            mmps = ps_mm.tile([128, 512], FP32, name="mmps",
                                              tag="s")
                            for kk in range(KK):
                                nc.tensor.matmul(
                                    mmps, xT_t[:, kk, c * 128:(c + 1) * 128],
                                    w_qkT[p][:, kk, :],
                                    start=(kk == 0), stop=(kk == KK - 1))
                            if p == 1:
                                nc.scalar.copy(
                                    qk_t[:, c, p * 512:(p + 1) * 512], mmps)
                            else:
                                nc.vector.tensor_copy(
                                    qk_t[:, c, p * 512:(p + 1) * 512], mmps)

                    if t == 0:
                        # v weight prep streams in under tile-0's qk compute
                        for m in range(12, 18):
                            prep_w_block(m)

                    # covariance + Gram: per head, q-side gives [Gq | C]
                    # (rhs = strided [q_h|k_h] view, 192 free), k-side gives
                    # Gk only (96 free). One psum bank per head. Runs before
                    # vT so the flush adds drain off DVE while the PE chews
                    # the vT matmuls (no DVE backlog at the next tile's qk).
                    for h in range(H):
                        cg_ps = ps_mm.tile([HD, 288], FP32, name="cg_ps",
                                           tag="s")
                        for c in range(CHUNKS):
                            qh = qk_t[:, c, HD * h:HD * (h + 1)]
                            kh = qk_t[:, c, C + HD * h:C + HD * (h + 1)]
                            qk_pair = qk_t[:, c, :].rearrange(
                                "p (j e) -> p j e", j=2)[:, :,
                                                         HD * h:HD * (h + 1)]
                            nc.tensor.matmul(
                                cg_ps[:, 0:192], qh, qk_pair,
                                start=(c == 0), stop=False)
                            nc.tensor.matmul(
                                cg_ps[:, 192:288], kh, kh,
                                start=False, stop=(c == CHUNKS - 1))
                        if t == 0:
                            nc.vector.tensor_copy(cg_accum[:, h, :], cg_ps)
                        else:
                            nc.vector.tensor_add(
                                cg_accum[:, h, :], cg_ps, cg_accum[:, h, :])

                    # vT = w_vT.T @ xT (feature-major) -> DRAM. The next
                    # tile's transposes are interleaved between vT groups so
                    # each xps->xT copy has ~2.5us of PE cover (no psum-
                    # rotation stall at the tile boundary).
                    t0 = t * TOK_TILE
                    if t + 1 < N_TILES:
                        xT_next = xtp.tile([128, KK, TOK_TILE], BF16,
                                           name="xT_t")
                    vt_sb = vtsb.tile([128, KK, TOK_TILE], BF16, name="vt_sb")
                    for m in range(KK):
                        vps = ps_mm.tile([128, TOK_TILE], FP32, name="vps",
                                         tag="s")
                        for kk in range(KK):
                            nc.tensor.matmul(
                                vps, w_vT[m][:, kk, :],
                                xT_t[:, kk, :],
                                start=(kk == 0), stop=(kk == KK - 1))
                        nc.scalar.copy(vt_sb[:, m, :], vps)
                        if t + 1 < N_TILES:
                            tr_group(xT_next, xcs, m)
                    nc.scalar.dma_start(
                        vT_dram[:, t0:t0 + TOK_TILE].rearrange(
                            "(s p) n -> p s n", p=128),
                        vt_sb)
                    if t == N_TILES - 1:
                        # Gram diagonals, hidden under the t7 vT matmuls.
                        # k-side first: the PE rk-broadcast path depends only
                        # on the k norms, so it starts while DVE still
                        # processes the q side.
                        for h in range(H):
                            nc.vector.tensor_tensor(
                                scr96, cg_accum[:, h, 192:288], identf,
                                mybir.AluOpType.mult)
                            nc.vector.reduce_sum(
                                sq[:, 1, h:h + 1], scr96,
                                axis=mybir.AxisListType.X)
                        for h in range(H):
                            nc.vector.tensor_tensor(
                                scr96, cg_accum[:, h, 0:HD], identf,
                                mybir.AluOpType.mult)
                            nc.vector.reduce_sum(
                                sq[:, 0, h:h + 1], scr96,
                                axis=mybir.AxisListType.X)
                    if t < 2:
                        # prefetch attn@v inputs for the phase-2/3 handoff
                        load_vt(t)
                    if t == 6:
                        # w_projT prep hides under tile-7 compute
                        prep_wproj()


        # ---------------- phase 3 pools; w_projT prep emitted first so the
        # PE has work while the DVE/ACT-heavy phase 2 chain runs ----------
        with tc.tile_pool(name="otp", bufs=2) as otp, \
             tc.tile_pool(name="yp", bufs=2) as yp, \
             tc.tile_pool(name="ps_o", bufs=2, space="PSUM") as ps_o, \
             tc.tile_pool(name="ps_y", bufs=4, space="PSUM") as ps_y:
            # ---------------- phase 2: norms + softmax ----------------
            # rsqrt(Gram diag) via bit-trick seed + 2 Newton steps, all on
            # DVE: keeps Sqrt off ACT so Exp/Copy share one act table set
            # (no LoadActFuncSet on the critical chain).
            yv = rsq_s[:, :, 0, :]
            av = rsq_s[:, :, 1, :]
            nc.vector.tensor_scalar(
                yv.bitcast(UINT32), sq.bitcast(UINT32), 1, None,
                mybir.AluOpType.logical_shift_right)
            nc.vector.tensor_tensor(yv.bitcast(UINT32), rsq_k,
                                    yv.bitcast(UINT32),
                                    mybir.AluOpType.subtract)
            for _ in range(2):
                nc.vector.tensor_tensor(av, yv, yv, mybir.AluOpType.mult)
                nc.vector.tensor_tensor(av, av, sq, mybir.AluOpType.mult)
                nc.vector.tensor_scalar(av, av, -0.5, 1.5,
                                        mybir.AluOpType.mult,
                                        mybir.AluOpType.add)
                nc.vector.tensor_tensor(yv, yv, av, mybir.AluOpType.mult)
            rnorm = yv
            rq = singles.tile([HD, H], FP32)
            nc.vector.tensor_tensor(rq, rnorm[:, 0, :], temp_all,
                                    mybir.AluOpType.mult)

            # rk to the free dim via PE: per-head single-row transposes land
            # rk rows at partition 0; concatenate to [1, 768], then one rank-1
            # broadcast matmul (ones[1,96].T @ rk_flat) fills all partitions.
            rk_flat = singles.tile([1, H * HD], FP32R)
            for h in range(H):
                rkt_ps = ps_tr.tile([1, HD], FP32, name="rkt_ps", tag="tr")
                nc.tensor.transpose(rkt_ps, rnorm[:, 1, h:h + 1], identf)
                nc.vector.tensor_copy(
                    rk_flat[:, HD * h:HD * (h + 1)], rkt_ps)
            attL = singles.tile([HD, H, HD], FP32)
            for half in range(2):
                rb_ps = ps_tr.tile([HD, 4 * HD], FP32, name="rb_ps", tag="tr")
                nc.tensor.matmul(
                    rb_ps, ones1,
                    rk_flat[:, half * 4 * HD:(half + 1) * 4 * HD],
                    start=True, stop=True)
                nc.vector.tensor_tensor(
                    attL[:, 4 * half:4 * (half + 1), :],
                    cg_accum[:, 4 * half:4 * (half + 1), HD:2 * HD],
                    rb_ps.rearrange("p (h e) -> p h e", h=4),
                    mybir.AluOpType.mult)

            # per-head softmax pipeline: logits = C * rq[d] * rk[e] * temp,
            # |C*rq*rk| <= 1 (normalized correlation) so exp needs no max
            # subtraction. Exp folds rq in as a per-partition scale; the
            # 1/sum normalization is folded into the otsb copies later, so
            # each head's transpose (and attn@v) can start right after its
            # exp instead of after a batched chain.
            sea = singles.tile([HD, H, 1], FP32)
            rsea = singles.tile([HD, H, 1], FP32)
            for h in range(H):
                nc.scalar.activation(attL[:, h, :], attL[:, h, :],
                                     mybir.ActivationFunctionType.Exp,
                                     scale=rq[:, h:h + 1])
                nc.vector.reduce_sum(sea[:, h, :], attL[:, h, :],
                                     axis=mybir.AxisListType.X)
                nc.vector.reciprocal(rsea[:, h, :], sea[:, h, :])
                atps = ps_tr.tile([HD, HD], FP32, name="atps", tag="tr")
                nc.tensor.transpose(atps, attL[:, h, :], identf)
                if h % 2 == 0:
                    nc.vector.tensor_copy(attnT[:, h, :], atps)
                else:
                    nc.scalar.copy(attnT[:, h, :], atps)

            # ---------------- phase 3: attn@v + proj, sw-pipelined --------
            # attn@v psum rows (96/head) are repacked into 128-row feature
            # blocks so proj contracts over 6 full blocks instead of 8 ragged
            # head slices.
            def attnv_stage(t):
                vT_t = vt_tiles.pop(t)
                otsb = otp.tile([128, KK, TOK_TILE], BF16, name="otsb")
                flip = 0
                for h in range(H):
                    ops_ = ps_o.tile([HD, TOK_TILE], FP32, name="ops_")
                    nc.tensor.matmul(ops_, attnT[:, h, :], vT_t[:, h, :],
                                     start=True, stop=True)
                    if h < 6:
                        pieces = [(h, 0, 0, HD)]
                    else:
                        pieces = [(3 * (h - 6) + u, 96, 32 * u, 32)
                                  for u in range(3)]
                    for (bb, pp, s0, ln) in pieces:
                        dst = otsb[pp:pp + ln, bb, :]
                        src = ops_[s0:s0 + ln, :]
                        sc = rsea[s0:s0 + ln, h, :]
                        if flip % 2 == 0:
                            nc.vector.tensor_scalar(dst, src, sc, None,
                                                    mybir.AluOpType.mult)
                        else:
                            nc.scalar.mul(dst, src, sc)
                        flip += 1
                if t + 2 < N_TILES:
                    load_vt(t + 2)
                return otsb

            def proj_stage(t, otsb):
                t0 = t * TOK_TILE
                last = t == N_TILES - 1
                y_t = yp.tile([128, CHUNKS, C], FP32, name="y_t")
                for c in range(CHUNKS):
                    for (off, width) in ((0, 512), (512, 256)):
                        yps = ps_y.tile([128, 512], FP32, name="yps")
                        for m in range(KK):
                            nc.tensor.matmul(
                                yps[:, :width],
                                otsb[:, m, c * 128:(c + 1) * 128],
                                w_projT[:, m, off:off + width],
                                start=(m == 0), stop=(m == KK - 1))
                        nc.vector.tensor_tensor(
                            y_t[:, c, off:off + width], yps[:, :width],
                            b_all[:, off:off + width], mybir.AluOpType.add)
                        if last:
                            nc.sync.dma_start(
                                out_d[t0 + c * 128:t0 + (c + 1) * 128,
                                      off:off + width],
                                y_t[:, c, off:off + width])
                    if not last:
                        nc.sync.dma_start(
                            out_d[t0 + c * 128:t0 + (c + 1) * 128, :],
                            y_t[:, c, :])

            pend = None
            for t in range(N_TILES):
                cur = attnv_stage(t)
                if pend is not None:
                    proj_stage(*pend)
                pend = (t, cur)
            proj_stage(*pend)


def _get_nc():
    global _CACHED_NC
    if _CACHED_NC is None:
        _CACHED_NC = build_nc()
    return _CACHED_NC


def kernel(x, w_qkv, temperature, w_proj, b_proj):
    nc = _get_nc()
    x = np.ascontiguousarray(np.asarray(x, dtype=np.float32))
    in_maps = []
    for b in range(8):
        in_maps.append({
            "x": x[b],
            "w_qkv": np.asarray(w_qkv, dtype=np.float32),
            "temperature": np.asarray(temperature, dtype=np.float32),
            "w_proj": np.asarray(w_proj, dtype=np.float32),
            "b_proj": np.asarray(b_proj, dtype=np.float32),
        })
    res = run_bass_kernel_spmd(nc, in_maps, core_ids=list(range(8)))
    return np.stack([r["out"] for r in res.results], axis=0)
